# revision 18
# baseline (speedup 1.0000x reference)
"""Raw-bass (manual semaphore) variant of the equivariant-linear kernel.

Math: per head h, out[b,:,h::8] = M_h^T @ x[b,:,h::8] with M_h the
512x512 3D-circulant generated from (basis@kernel)[:,h]; only 4 distinct
128x128 blocks (d = (kc-mc) mod 4). One head per NeuronCore.

Layouts (per core):
  x16 (4 tb, 128, 2048) fp16 : row p = [kc0|kc1|kc2|kc3] tokens of block tb
  w16 (128, 512) fp16        : row p = [d0|d1|d2|d3]
  o16 (4 mc, 4 tb, 128, 512) fp16
4KB-per-partition-row input DMAs (DMA cost is per packet, not per byte).
"""

import os
from contextlib import ExitStack

import numpy as np

NUM_HEADS = 8
BATCH = 32
SEQ = 512
CHAN = 512
CH = CHAN // NUM_HEADS
P = 128
NKC = 4
NMC = 4
TOK = BATCH * CH
NTB = 4
N_WARM = 12

LAST_RESULT = None
_BASS_CACHE = None


def _build_bass():
    import concourse.bass as bass
    import concourse.mybir as mybir

    fp16 = mybir.dt.float16
    fp32 = mybir.dt.float32

    nc = bass.Bass()

    x_d = nc.dram_tensor("x16", [NTB, P, NKC * 512], fp16, kind="ExternalInput")
    w_d = nc.dram_tensor("w16", [P, 4 * P], fp16, kind="ExternalInput")
    o_d = nc.dram_tensor("o16", [NMC, NTB, P, 512], fp16, kind="ExternalOutput")

    ctx = ExitStack()
    with ctx:
        XT = [
            ctx.enter_context(nc.sbuf_tensor(f"x_{tb}", [P, NKC * 512], fp16))
            for tb in range(NTB)
        ]
        warm_w = ctx.enter_context(nc.sbuf_tensor("warm_w", [P, 512], fp16))
        WT = ctx.enter_context(nc.sbuf_tensor("w_all", [P, 4 * P], fp16))
        OT = [
            ctx.enter_context(nc.sbuf_tensor(f"ot_{i}", [P, 512], fp16))
            for i in range(16)
        ]
        PS = [
            ctx.enter_context(nc.psum_tensor(f"ps_{i}", [P, 512], fp32))
            for i in range(8)
        ]

        sem_mm = ctx.enter_context(nc.semaphore("mm"))
        sem_cp = ctx.enter_context(nc.semaphore("cp"))
        sem_cpa = ctx.enter_context(nc.semaphore("cpa"))
        sem_od = ctx.enter_context(nc.semaphore("od"))
        sem_od_sw = ctx.enter_context(nc.semaphore("od_sw"))
        sem_warm = ctx.enter_context(nc.semaphore("warm"))
        sem_wa = ctx.enter_context(nc.semaphore("in_wa"))
        sem_wb = ctx.enter_context(nc.semaphore("in_wb"))
        sem_x0t = ctx.enter_context(nc.semaphore("in_x0t"))
        sem_x0b = ctx.enter_context(nc.semaphore("in_x0b"))
        sem_x1 = ctx.enter_context(nc.semaphore("in_x1"))
        sem_x2 = ctx.enter_context(nc.semaphore("in_x2"))
        sem_x3 = ctx.enter_context(nc.semaphore("in_x3"))

        # matmul schedule: (tb, d, mc, start, stop). d-major (weight reuse)
        # except the last tb, which is mc-major so the final psum groups
        # retire (and copy out) early.
        mm_order = []
        for tb in (0, 1, 2):
            for d in range(4):
                for mc in range(NMC):
                    mm_order.append((tb, d, mc, d == 0, d == 3))
        for mc in range(NMC):
            for d in range(4):
                mm_order.append((3, d, mc, d == 0, d == 3))

        def ps_tile(tb, mc):
            return PS[(tb % 2) * 4 + mc]

        grp_done = {}
        ngrp = 0
        for tb, d, mc, start, stop in mm_order:
            if stop:
                ngrp += 1
                grp_done[(tb, mc)] = ngrp

        # out-DMA queue assignment: early blocks round-robin all 3 queues,
        # final block only on the low-latency HWDGE queues.
        def out_queue(i):
            if i >= 12:
                return ("sync", "scalar")[i % 2]
            return ("gpsimd", "sync", "scalar")[i % 3]

        # psum->sbuf copies split across DVE and ACT, each with its own
        # ordered count semaphore. Last tb: ACT (2x slower) takes the early
        # groups, DVE the final two, so the tail is short.
        def copy_engine(tb, mc):
            if tb == NTB - 1:
                return "dve"
            return "dve" if mc % 2 == 0 else "act"

        cp_count = {}
        ndve = nact = 0
        for tb in range(NTB):
            for mc in range(NMC):
                if copy_engine(tb, mc) == "dve":
                    ndve += 1
                    cp_count[(tb, mc)] = ("dve", ndve)
                else:
                    nact += 1
                    cp_count[(tb, mc)] = ("act", nact)

        def wait_copy(eng, tb, mc):
            which, cnt = cp_count[(tb, mc)]
            eng.wait_ge(sem_cp if which == "dve" else sem_cpa, cnt)

        with nc.Block() as block:

            @block.sync
            def _(sync):
                sync.dma_start(WT[:64], w_d[:64]).then_inc(sem_wa, 16)
                sync.dma_start(XT[0][:64], x_d[0][:64]).then_inc(sem_x0t, 16)
                sync.dma_start(XT[2][:], x_d[2]).then_inc(sem_x2, 16)
                for tb in range(NTB):
                    for mc in range(NMC):
                        i = tb * 4 + mc
                        if out_queue(i) == "sync":
                            wait_copy(sync, tb, mc)
                            sync.dma_start(o_d[mc, tb], OT[i][:]).then_inc(sem_od, 16)

            @block.scalar
            def _(scalar):
                scalar.dma_start(WT[64:], w_d[64:]).then_inc(sem_wb, 16)
                scalar.dma_start(XT[0][64:], x_d[0][64:]).then_inc(sem_x0b, 16)
                scalar.dma_start(XT[1][:], x_d[1]).then_inc(sem_x1, 16)
                scalar.wait_ge(sem_x1, 16)
                scalar.dma_start(XT[3][:], x_d[3]).then_inc(sem_x3, 16)
                for tb in range(NTB):
                    for mc in range(NMC):
                        i = tb * 4 + mc
                        if copy_engine(tb, mc) == "act":
                            scalar.wait_ge(sem_mm, grp_done[(tb, mc)])
                            nc.scalar.copy(OT[i][:], ps_tile(tb, mc)[:]).then_inc(
                                sem_cpa, 1
                            )
                        if out_queue(i) == "scalar":
                            wait_copy(scalar, tb, mc)
                            scalar.dma_start(o_d[mc, tb], OT[i][:]).then_inc(
                                sem_od, 16
                            )

            @block.gpsimd
            def _(gpsimd):
                gpsimd.memset(warm_w[:], 0.0).then_inc(sem_warm, 1)
                for tb in range(NTB):
                    for mc in range(NMC):
                        i = tb * 4 + mc
                        if out_queue(i) == "gpsimd":
                            wait_copy(gpsimd, tb, mc)
                            gpsimd.dma_start(o_d[mc, tb], OT[i][:]).then_inc(
                                sem_od_sw, 16
                            )

            @block.tensor
            def _(tensor):
                # HAM warm-up: full-width matmuls (N=512) on a zeroed tile
                # so the clock-gate sees real PE activity while inputs stream.
                tensor.wait_ge(sem_warm, 1)
                for _ in range(N_WARM):
                    nc.tensor.matmul(
                        PS[7][:], warm_w[:, :P], warm_w[:], start=True, stop=True,
                        skip_group_check=True,
                    )
                tensor.wait_ge(sem_wa, 16)
                tensor.wait_ge(sem_wb, 16)
                tensor.wait_ge(sem_x0t, 16)
                tensor.wait_ge(sem_x0b, 16)
                xsem = {1: sem_x1, 2: sem_x2, 3: sem_x3}
                cur_tb = 0
                for tb, d, mc, start, stop in mm_order:
                    kc = (mc + d) % NKC
                    if tb != cur_tb:
                        tensor.wait_ge(xsem[tb], 16)
                        if tb >= 2:
                            # WAR: psum banks reused from tb-2; count copies
                            # of tb-2 done per engine
                            ndv = sum(1 for t in range(tb - 1) for m in range(NMC)
                                      if copy_engine(t, m) == "dve")
                            nac = sum(1 for t in range(tb - 1) for m in range(NMC)
                                      if copy_engine(t, m) == "act")
                            tensor.wait_ge(sem_cp, ndv)
                            tensor.wait_ge(sem_cpa, nac)
                        cur_tb = tb
                    mm = nc.tensor.matmul(
                        ps_tile(tb, mc)[:],
                        WT[:, d * P:(d + 1) * P],
                        XT[tb][:, kc * 512:(kc + 1) * 512],
                        start=start,
                        stop=stop,
                        skip_group_check=True,
                    )
                    if stop:
                        mm.then_inc(sem_mm, 1)

            @block.vector
            def _(vector):
                for tb in range(NTB):
                    for mc in range(NMC):
                        i = tb * 4 + mc
                        if copy_engine(tb, mc) == "dve":
                            vector.wait_ge(sem_mm, grp_done[(tb, mc)])
                            nc.vector.tensor_copy(
                                OT[i][:], ps_tile(tb, mc)[:]
                            ).then_inc(sem_cp, 1)

    return nc


def _weight_tiles(kexp_h):
    w3 = kexp_h.reshape(8, 8, 8)
    p = np.arange(P)
    m = np.arange(P)
    dj = ((p[:, None] // 8) % 8 - (m[None, :] // 8) % 8) % 8
    dk = (p[:, None] % 8 - m[None, :] % 8) % 8
    tiles = np.empty((4, P, P), np.float32)
    for d in range(4):
        di = (2 * d + p[:, None] // 64 - m[None, :] // 64) % 8
        tiles[d] = w3[di, dj, dk]
    return tiles


def _host_prep(x, kexp, h):
    xh = x[:, :, h::NUM_HEADS]  # (32, 512, 64)
    x_dev = (
        xh.transpose(1, 0, 2)        # (g'', b, c)
        .reshape(NKC, P, NTB, 512)   # (kc, p, tb, n)
        .transpose(2, 1, 0, 3)       # (tb, p, kc, n)
        .reshape(NTB, P, NKC * 512)
        .astype(np.float16)
    )
    w_dev = (
        _weight_tiles(kexp[:, h])    # (d, p, m)
        .transpose(1, 0, 2)          # (p, d, m)
        .reshape(P, 4 * P)
        .astype(np.float16)
    )
    return np.ascontiguousarray(x_dev), np.ascontiguousarray(w_dev)


def kernel(x, basis, kernel):
    global LAST_RESULT, _BASS_CACHE
    from concourse.bass_utils import run_bass_kernel_spmd

    x = np.ascontiguousarray(np.asarray(x, dtype=np.float32))
    kexp = np.asarray(basis, np.float32) @ np.asarray(kernel, np.float32)

    in_maps = []
    for h in range(NUM_HEADS):
        x_dev, w_dev = _host_prep(x, kexp, h)
        in_maps.append({"x16": x_dev, "w16": w_dev})

    if _BASS_CACHE is None:
        _BASS_CACHE = _build_bass()
    nc = _BASS_CACHE

    LAST_RESULT = run_bass_kernel_spmd(
        nc,
        in_maps,
        core_ids=list(range(NUM_HEADS)),
        trace=bool(int(os.environ.get("KERNEL_TRACE", "0"))),
    )

    out = np.empty((BATCH, SEQ, CHAN), np.float32)
    for h in range(NUM_HEADS):
        o_dev = LAST_RESULT.results[h]["o16"].astype(np.float32)  # (mc, tb, m, n)
        out_h = o_dev.transpose(0, 2, 1, 3).reshape(SEQ, TOK)
        out[:, :, h::NUM_HEADS] = out_h.reshape(SEQ, BATCH, CH).transpose(1, 0, 2)
    return out


# revision 19
# speedup vs baseline: 1.0151x; 1.0151x over previous
"""Raw-bass (manual semaphore) variant of the equivariant-linear kernel.

Math: per head h, out[b,:,h::8] = M_h^T @ x[b,:,h::8] with M_h the
512x512 3D-circulant generated from (basis@kernel)[:,h]; only 4 distinct
128x128 blocks (d = (kc-mc) mod 4). One head per NeuronCore.

Layouts (per core):
  x16 (4 tb, 128, 2048) fp16 : row p = [kc0|kc1|kc2|kc3] tokens of block tb
  w16 (128, 512) fp16        : row p = [d0|d1|d2|d3]
  o16 (4 mc, 4 tb, 128, 512) fp16
4KB-per-partition-row input DMAs (DMA cost is per packet, not per byte).
"""

import os
from contextlib import ExitStack

import numpy as np

NUM_HEADS = 8
BATCH = 32
SEQ = 512
CHAN = 512
CH = CHAN // NUM_HEADS
P = 128
NKC = 4
NMC = 4
TOK = BATCH * CH
NTB = 4
N_WARM = 10

LAST_RESULT = None
_BASS_CACHE = None


def _build_bass():
    import concourse.bass as bass
    import concourse.mybir as mybir

    fp16 = mybir.dt.float16
    fp32 = mybir.dt.float32

    nc = bass.Bass()

    x_d = nc.dram_tensor("x16", [NTB, P, NKC * 512], fp16, kind="ExternalInput")
    w_d = nc.dram_tensor("w16", [P, 4 * P], fp16, kind="ExternalInput")
    o_d = nc.dram_tensor("o16", [NMC, NTB, P, 512], fp16, kind="ExternalOutput")

    ctx = ExitStack()
    with ctx:
        XT = [
            ctx.enter_context(nc.sbuf_tensor(f"x_{tb}", [P, NKC * 512], fp16))
            for tb in range(NTB)
        ]
        warm_w = ctx.enter_context(nc.sbuf_tensor("warm_w", [P, 512], fp16))
        WT = ctx.enter_context(nc.sbuf_tensor("w_all", [P, 4 * P], fp16))
        OT = [
            ctx.enter_context(nc.sbuf_tensor(f"ot_{i}", [P, 512], fp16))
            for i in range(16)
        ]
        PS = [
            ctx.enter_context(nc.psum_tensor(f"ps_{i}", [P, 512], fp32))
            for i in range(8)
        ]

        sem_mm = ctx.enter_context(nc.semaphore("mm"))
        sem_cp = ctx.enter_context(nc.semaphore("cp"))
        sem_cpa = ctx.enter_context(nc.semaphore("cpa"))
        sem_od = ctx.enter_context(nc.semaphore("od"))
        sem_od_sw = ctx.enter_context(nc.semaphore("od_sw"))
        sem_warm = ctx.enter_context(nc.semaphore("warm"))
        sem_wa = ctx.enter_context(nc.semaphore("in_wa"))
        sem_wb = ctx.enter_context(nc.semaphore("in_wb"))
        sem_x0t = ctx.enter_context(nc.semaphore("in_x0t"))
        sem_x0b = ctx.enter_context(nc.semaphore("in_x0b"))
        sem_x1 = ctx.enter_context(nc.semaphore("in_x1"))
        sem_x2 = ctx.enter_context(nc.semaphore("in_x2"))
        sem_x3 = ctx.enter_context(nc.semaphore("in_x3"))

        # matmul schedule: (tb, d, mc, start, stop). d-major (weight reuse)
        # except the last tb, which is mc-major so the final psum groups
        # retire (and copy out) early.
        mm_order = []
        for tb in (0, 1, 2):
            for d in range(4):
                for mc in range(NMC):
                    mm_order.append((tb, d, mc, d == 0, d == 3))
        for mc in range(NMC):
            for d in range(4):
                mm_order.append((3, d, mc, d == 0, d == 3))

        def ps_tile(tb, mc):
            return PS[(tb % 2) * 4 + mc]

        grp_done = {}
        ngrp = 0
        for tb, d, mc, start, stop in mm_order:
            if stop:
                ngrp += 1
                grp_done[(tb, mc)] = ngrp

        # out-DMA queue assignment: early blocks round-robin all 3 queues,
        # final block only on the low-latency HWDGE queues.
        def out_queue(i):
            if i >= 12:
                return ("sync", "scalar")[i % 2]
            return ("gpsimd", "sync", "scalar")[i % 3]

        # psum->sbuf copies split across DVE and ACT, each with its own
        # ordered count semaphore. Last tb: ACT (2x slower) takes the early
        # groups, DVE the final two, so the tail is short.
        def copy_engine(tb, mc):
            if tb == NTB - 1:
                return "dve"
            return "dve" if mc % 2 == 0 else "act"

        cp_count = {}
        ndve = nact = 0
        for tb in range(NTB):
            for mc in range(NMC):
                if copy_engine(tb, mc) == "dve":
                    ndve += 1
                    cp_count[(tb, mc)] = ("dve", ndve)
                else:
                    nact += 1
                    cp_count[(tb, mc)] = ("act", nact)

        def wait_copy(eng, tb, mc):
            which, cnt = cp_count[(tb, mc)]
            eng.wait_ge(sem_cp if which == "dve" else sem_cpa, cnt)

        with nc.Block() as block:

            @block.sync
            def _(sync):
                sync.dma_start(WT[:64], w_d[:64]).then_inc(sem_wa, 16)
                sync.dma_start(XT[0][:64], x_d[0][:64]).then_inc(sem_x0t, 16)
                sync.dma_start(XT[2][:], x_d[2]).then_inc(sem_x2, 16)
                for tb in range(NTB):
                    for mc in range(NMC):
                        i = tb * 4 + mc
                        if out_queue(i) == "sync":
                            wait_copy(sync, tb, mc)
                            sync.dma_start(o_d[mc, tb], OT[i][:]).then_inc(sem_od, 16)

            @block.scalar
            def _(scalar):
                scalar.dma_start(WT[64:], w_d[64:]).then_inc(sem_wb, 16)
                scalar.dma_start(XT[0][64:], x_d[0][64:]).then_inc(sem_x0b, 16)
                scalar.dma_start(XT[1][:], x_d[1]).then_inc(sem_x1, 16)
                scalar.wait_ge(sem_x1, 16)
                scalar.dma_start(XT[3][:], x_d[3]).then_inc(sem_x3, 16)
                for tb in range(NTB):
                    for mc in range(NMC):
                        i = tb * 4 + mc
                        if copy_engine(tb, mc) == "act":
                            scalar.wait_ge(sem_mm, grp_done[(tb, mc)])
                            nc.scalar.copy(OT[i][:], ps_tile(tb, mc)[:]).then_inc(
                                sem_cpa, 1
                            )
                        if out_queue(i) == "scalar":
                            wait_copy(scalar, tb, mc)
                            scalar.dma_start(o_d[mc, tb], OT[i][:]).then_inc(
                                sem_od, 16
                            )

            @block.gpsimd
            def _(gpsimd):
                gpsimd.memset(warm_w[:], 0.0).then_inc(sem_warm, 1)
                for tb in range(NTB):
                    for mc in range(NMC):
                        i = tb * 4 + mc
                        if out_queue(i) == "gpsimd":
                            wait_copy(gpsimd, tb, mc)
                            gpsimd.dma_start(o_d[mc, tb], OT[i][:]).then_inc(
                                sem_od_sw, 16
                            )

            @block.tensor
            def _(tensor):
                # HAM warm-up: full-width matmuls (N=512) on a zeroed tile
                # so the clock-gate sees real PE activity while inputs stream.
                tensor.wait_ge(sem_warm, 1)
                for _ in range(N_WARM):
                    nc.tensor.matmul(
                        PS[7][:], warm_w[:, :P], warm_w[:], start=True, stop=True,
                        skip_group_check=True,
                    )
                tensor.wait_ge(sem_wa, 16)
                tensor.wait_ge(sem_wb, 16)
                tensor.wait_ge(sem_x0t, 16)
                tensor.wait_ge(sem_x0b, 16)
                xsem = {1: sem_x1, 2: sem_x2, 3: sem_x3}
                cur_tb = 0
                for tb, d, mc, start, stop in mm_order:
                    kc = (mc + d) % NKC
                    if tb != cur_tb:
                        tensor.wait_ge(xsem[tb], 16)
                        if tb >= 2:
                            # WAR: psum banks reused from tb-2; count copies
                            # of tb-2 done per engine
                            ndv = sum(1 for t in range(tb - 1) for m in range(NMC)
                                      if copy_engine(t, m) == "dve")
                            nac = sum(1 for t in range(tb - 1) for m in range(NMC)
                                      if copy_engine(t, m) == "act")
                            tensor.wait_ge(sem_cp, ndv)
                            tensor.wait_ge(sem_cpa, nac)
                        cur_tb = tb
                    mm = nc.tensor.matmul(
                        ps_tile(tb, mc)[:],
                        WT[:, d * P:(d + 1) * P],
                        XT[tb][:, kc * 512:(kc + 1) * 512],
                        start=start,
                        stop=stop,
                        skip_group_check=True,
                    )
                    if stop:
                        mm.then_inc(sem_mm, 1)

            @block.vector
            def _(vector):
                for tb in range(NTB):
                    for mc in range(NMC):
                        i = tb * 4 + mc
                        if copy_engine(tb, mc) == "dve":
                            vector.wait_ge(sem_mm, grp_done[(tb, mc)])
                            nc.vector.tensor_copy(
                                OT[i][:], ps_tile(tb, mc)[:]
                            ).then_inc(sem_cp, 1)

    return nc


def _weight_tiles(kexp_h):
    w3 = kexp_h.reshape(8, 8, 8)
    p = np.arange(P)
    m = np.arange(P)
    dj = ((p[:, None] // 8) % 8 - (m[None, :] // 8) % 8) % 8
    dk = (p[:, None] % 8 - m[None, :] % 8) % 8
    tiles = np.empty((4, P, P), np.float32)
    for d in range(4):
        di = (2 * d + p[:, None] // 64 - m[None, :] // 64) % 8
        tiles[d] = w3[di, dj, dk]
    return tiles


def _host_prep(x, kexp, h):
    xh = x[:, :, h::NUM_HEADS]  # (32, 512, 64)
    x_dev = (
        xh.transpose(1, 0, 2)        # (g'', b, c)
        .reshape(NKC, P, NTB, 512)   # (kc, p, tb, n)
        .transpose(2, 1, 0, 3)       # (tb, p, kc, n)
        .reshape(NTB, P, NKC * 512)
        .astype(np.float16)
    )
    w_dev = (
        _weight_tiles(kexp[:, h])    # (d, p, m)
        .transpose(1, 0, 2)          # (p, d, m)
        .reshape(P, 4 * P)
        .astype(np.float16)
    )
    return np.ascontiguousarray(x_dev), np.ascontiguousarray(w_dev)


def kernel(x, basis, kernel):
    global LAST_RESULT, _BASS_CACHE
    from concourse.bass_utils import run_bass_kernel_spmd

    x = np.ascontiguousarray(np.asarray(x, dtype=np.float32))
    kexp = np.asarray(basis, np.float32) @ np.asarray(kernel, np.float32)

    in_maps = []
    for h in range(NUM_HEADS):
        x_dev, w_dev = _host_prep(x, kexp, h)
        in_maps.append({"x16": x_dev, "w16": w_dev})

    if _BASS_CACHE is None:
        _BASS_CACHE = _build_bass()
    nc = _BASS_CACHE

    LAST_RESULT = run_bass_kernel_spmd(
        nc,
        in_maps,
        core_ids=list(range(NUM_HEADS)),
        trace=bool(int(os.environ.get("KERNEL_TRACE", "0"))),
    )

    out = np.empty((BATCH, SEQ, CHAN), np.float32)
    for h in range(NUM_HEADS):
        o_dev = LAST_RESULT.results[h]["o16"].astype(np.float32)  # (mc, tb, m, n)
        out_h = o_dev.transpose(0, 2, 1, 3).reshape(SEQ, TOK)
        out[:, :, h::NUM_HEADS] = out_h.reshape(SEQ, BATCH, CH).transpose(1, 0, 2)
    return out
